# revision 10
# baseline (speedup 1.0000x reference)
"""MoE (MiMoV2 FlashMoE) Trainium2 kernel: expert-parallel over 8 NeuronCores.

Strategy:
  Phase 1 (device): router — logits = x @ w_router.T computed in fp32,
    top-4 selection via exact max/mask iterations on logits, combine
    weights = sigmoid(logit) normalized over the selected 4. Each core
    handles T/8 = 512 tokens. Output: dense combine matrix [T, E]
    (4 nonzeros per row).
  Host: compaction — per-expert token index lists from combine > 0
    (pure data movement). Experts are assigned to (core, slot) by
    descending load so that slot s has capacity C_s = max load of its
    8 experts; per-core padded columns drop from 4*Cmax to sum(C_s).
  Phase 2 (device): experts — 4 expert slots per core, all matmuls in
    bf16 (1 cycle/row on PE, half the DMA traffic of f32).
    G = Wg @ X, U = Wu @ X, h = silu(G)*U*combine (Act+DVE, bf16),
    Y^T = Wd @ h, PSUM f32 -> bf16 staging -> one output DMA per slot.
  Host: scatter-add per-expert outputs into y [T, H] (unique indices
    per expert).
"""
import math
import numpy as np
from contextlib import ExitStack

import ml_dtypes

import concourse.bass as bass
import concourse.mybir as mybir
import concourse.tile as tile
from concourse import bacc
from concourse.bass_utils import run_bass_kernel_spmd

F32 = mybir.dt.float32
F32R = mybir.dt.float32r
BF16 = mybir.dt.bfloat16
NP_BF16 = ml_dtypes.bfloat16

# Problem shapes (hardcoded per contract)
E = 32          # experts
TOPK = 4
H = 1024        # hidden
I = 768         # intermediate
B, S = 2, 2048
T = B * S       # 4096 tokens
NCORES = 8
SLOTS = E // NCORES  # expert slots per core = 4
TPC = T // NCORES    # router tokens per core = 512
KH = H // 128        # 8 contraction chunks over H
KI = I // 128        # 6 contraction chunks over I

_program_cache = {}


def _ctiles(C):
    """Split C into near-equal column tiles, each <= 512 (PSUM bank).
    Measured on HW: near-equal halves beat [512, remainder] tiling."""
    n = max(1, math.ceil(C / 512))
    base = C // n
    rem = C - base * n
    sizes = [base + (1 if i < rem else 0) for i in range(n)]
    out, off = [], 0
    for s in sizes:
        out.append((off, s))
        off += s
    return out


def build_router(reps=1):
    """Per-core: logits = x @ w_router.T via PE with TOKENS stationary
    (x chunk [128k, 128toks] stationary, w_router [128k, E] moving), so the
    PSUM output is already [tokens, E] — no transposes needed. Then a
    batched top-4 + combine-weight computation on a single [128, 4, E]
    tile. Selection compares exact fp32 logits."""
    nc = bacc.Bacc()
    NT = TPC // 128  # 4 token tiles
    xTc = nc.dram_tensor("xTc", [H, TPC], F32, kind="ExternalInput")
    wrT = nc.dram_tensor("wrT", [H, E], F32, kind="ExternalInput")
    comb_out = nc.dram_tensor("comb", [128, NT, E], F32, kind="ExternalOutput")
    with ExitStack() as ctx:
        tc = ctx.enter_context(tile.TileContext(nc))
        sb = ctx.enter_context(tc.tile_pool(name="sb", bufs=1))
        work = ctx.enter_context(tc.tile_pool(name="work", bufs=2))
        ps = ctx.enter_context(tc.tile_pool(name="ps", bufs=2, space="PSUM"))

        xr = sb.tile([128, KH, TPC], F32)
        wr = sb.tile([128, KH, E], F32)
        for k in range(KH):
            nc.sync.dma_start(out=xr[:, k, :], in_=xTc[k * 128:(k + 1) * 128, :])
            nc.scalar.dma_start(out=wr[:, k, :], in_=wrT[k * 128:(k + 1) * 128, :])

        for _ in range(reps):
            lt = work.tile([128, NT, E], F32)
            for t in range(NT):
                lg_p = ps.tile([128, E], F32, tag="lg", name="lg_p")
                for k in range(KH):
                    nc.tensor.matmul(lg_p, xr[:, k, t * 128:(t + 1) * 128],
                                     wr[:, k, :],
                                     start=(k == 0), stop=(k == KH - 1))
                if t % 2 == 0:
                    nc.scalar.activation(lt[:, t, :], lg_p,
                                         mybir.ActivationFunctionType.Copy)
                else:
                    nc.vector.tensor_copy(lt[:, t, :], lg_p)
            # batched top-4: find 4th max per token via iterative masking
            cur = work.tile([128, NT, E], F32)
            nc.vector.tensor_copy(cur, lt)
            m = work.tile([128, NT, 1], F32)
            ge = work.tile([128, NT, E], F32)
            for _k in range(TOPK - 1):
                nc.vector.reduce_max(m, cur, axis=mybir.AxisListType.X)
                nc.vector.tensor_tensor(ge, cur, m.broadcast_to((128, NT, E)),
                                        op=mybir.AluOpType.is_ge)
                nc.vector.scalar_tensor_tensor(cur, ge, -1e30, cur,
                                               op0=mybir.AluOpType.mult,
                                               op1=mybir.AluOpType.add)
            nc.vector.reduce_max(m, cur, axis=mybir.AxisListType.X)
            # sel = (logits >= 4th max), combine = sel*sigmoid normalized
            sel = work.tile([128, NT, E], F32)
            nc.vector.tensor_tensor(sel, lt, m.broadcast_to((128, NT, E)),
                                    op=mybir.AluOpType.is_ge)
            sig = work.tile([128, NT, E], F32)
            nc.scalar.activation(sig, lt, mybir.ActivationFunctionType.Sigmoid)
            wsel = work.tile([128, NT, E], F32)
            nc.vector.tensor_mul(wsel, sel, sig)
            ssum = work.tile([128, NT, 1], F32)
            nc.vector.reduce_sum(ssum, wsel, axis=mybir.AxisListType.X)
            nc.vector.tensor_scalar_add(ssum, ssum, 1e-20)
            rsum = work.tile([128, NT, 1], F32)
            nc.vector.reciprocal(rsum, ssum)
            ct = work.tile([128, NT, E], F32)
            nc.vector.tensor_tensor(ct, wsel, rsum.broadcast_to((128, NT, E)),
                                    op=mybir.AluOpType.mult)
            nc.sync.dma_start(out=comb_out[:], in_=ct)
    nc.finalize()
    return nc


def build_experts(CS, reps=1):
    """Expert MLP kernel, bf16. CS = per-slot capacities (build constants).
    Per-core inputs (tile-exact layouts, all contiguous DMA):
      xg  [128, KH, CT]             bf16  xg[p,k,off_s+c] = x[tok_{s,c}, k*128+p]
      wgu [SLOTS, 128, KI, 2, KH, 128] bf16
          wgu[s,p,m,0,k,i] = w_gate[e_s, m*128+i, k*128+p]; [..,1,..] = w_up
      wd  [SLOTS, 128, KH, KI, 128] bf16  wd[s,p,h,k,o] = w_down[e_s, h*128+o, k*128+p]
      cw  [1, CT]                   bf16  combine weights (0 on padding)
    Output: yg [128, KH, CT] bf16, yg[p,h,off_s+c] = y^T[h*128+p, c]
    (combine-weighted, transposed)."""
    CS = tuple(CS)
    CT = sum(CS)
    CMAX = max(CS)
    offs = [0]
    for c in CS[:-1]:
        offs.append(offs[-1] + c)

    nc = bacc.Bacc()
    xgf = nc.dram_tensor("xg", [128, KH, CT], BF16, kind="ExternalInput")
    wgu = nc.dram_tensor("wgu", [SLOTS, 128, KI, 2, KH, 128], BF16,
                         kind="ExternalInput")
    wdd = nc.dram_tensor("wd", [SLOTS, 128, KH, KI, 128], BF16,
                         kind="ExternalInput")
    cwf = nc.dram_tensor("cw", [1, CT], BF16, kind="ExternalInput")
    ygf = nc.dram_tensor("yg", [128, KH, CT], BF16, kind="ExternalOutput")
    warm_out = nc.dram_tensor("warm", [128, 1], F32, kind="ExternalOutput")

    with ExitStack() as ctx:
        tc = ctx.enter_context(tile.TileContext(nc))
        const = ctx.enter_context(tc.tile_pool(name="const", bufs=1))
        xgp = ctx.enter_context(tc.tile_pool(name="xgp", bufs=2))
        wgp = ctx.enter_context(tc.tile_pool(name="wgp", bufs=2))
        wdp = ctx.enter_context(tc.tile_pool(name="wdp", bufs=2))
        hp = ctx.enter_context(tc.tile_pool(name="hp", bufs=2))
        msc = ctx.enter_context(tc.tile_pool(name="msc", bufs=3))
        outp = ctx.enter_context(tc.tile_pool(name="outp", bufs=2))
        ps_gu = ctx.enter_context(tc.tile_pool(name="ps_gu", bufs=2, space="PSUM"))
        ps_d = ctx.enter_context(tc.tile_pool(name="ps_d", bufs=2, space="PSUM"))

        cwb = const.tile([128, CT], BF16, tag="cw")
        nc.scalar.dma_start(out=cwb, in_=cwf[0:1, :].partition_broadcast(128))

        # PE warm-up: keep TensorE busy while the first weight/activation
        # DMAs land, so the HAM clock-gate releases (1.2 -> 2.4 GHz) before
        # real matmuls start. Results are dumped to a debug output.
        wtile = const.tile([128, 512], F32R, tag="warm")
        nc.vector.memset(wtile.bitcast(F32), 0.0)
        wps = ps_d.tile([128, 512], F32, tag="warmp")
        for wi in range(6):
            nc.tensor.matmul(wps, wtile[:, :128], wtile,
                             start=(wi == 0), stop=(wi == 5))
        wres = const.tile([128, 1], F32, tag="warmres")
        nc.vector.tensor_copy(wres, wps[:, 0:1])
        nc.gpsimd.dma_start(out=warm_out[:], in_=wres)

        SILU = mybir.ActivationFunctionType.Silu
        COPY = mybir.ActivationFunctionType.Copy
        for _ in range(reps):
            for s in range(SLOTS):
                C, off = CS[s], offs[s]
                cts = _ctiles(C)
                xg_t = xgp.tile([128, KH, CMAX], BF16, tag="xg")
                wgu_t = wgp.tile([128, KI, 2, KH, 128], BF16, tag="wgu")
                wd_t = wdp.tile([128, KH, KI, 128], BF16, tag="wd")
                nc.sync.dma_start(out=wgu_t[:, :, 0], in_=wgu[s, :, :, 0])
                nc.scalar.dma_start(out=wgu_t[:, :, 1], in_=wgu[s, :, :, 1])
                nc.sync.dma_start(out=xg_t[:, :, :C], in_=xgf[:, :, off:off + C])
                nc.scalar.dma_start(out=wd_t, in_=wdd[s])

                h_t = hp.tile([128, KI, CMAX], BF16, tag="h")
                for m in range(KI):
                    for ci, (c0, cn) in enumerate(cts):
                        gp = ps_gu.tile([128, 512], F32, tag="gp")
                        for k in range(KH):
                            nc.tensor.matmul(gp[:, :cn], wgu_t[:, m, 0, k, :],
                                             xg_t[:, k, c0:c0 + cn],
                                             start=(k == 0), stop=(k == KH - 1))
                        up = ps_gu.tile([128, 512], F32, tag="up")
                        for k in range(KH):
                            nc.tensor.matmul(up[:, :cn], wgu_t[:, m, 1, k, :],
                                             xg_t[:, k, c0:c0 + cn],
                                             start=(k == 0), stop=(k == KH - 1))
                        sg = msc.tile([128, 512], BF16, tag="sg")
                        nc.scalar.activation(sg[:, :cn], gp[:, :cn], SILU)
                        t1 = msc.tile([128, 512], BF16, tag="t1")
                        nc.vector.tensor_mul(t1[:, :cn], sg[:, :cn], up[:, :cn])
                        nc.vector.tensor_mul(h_t[:, m, c0:c0 + cn], t1[:, :cn],
                                             cwb[:, off + c0:off + c0 + cn])
                yo_t = outp.tile([128, KH, CMAX], BF16, tag="yo")
                for hh in range(KH):
                    for ci, (c0, cn) in enumerate(cts):
                        yp = ps_d.tile([128, 512], F32, tag="yp")
                        for k in range(KI):
                            nc.tensor.matmul(yp[:, :cn], wd_t[:, hh, k, :],
                                             h_t[:, k, c0:c0 + cn],
                                             start=(k == 0), stop=(k == KI - 1))
                        # alternate the PSUM->SBUF bf16 copy between Act/DVE
                        if (hh * len(cts) + ci) % 2 == 0:
                            nc.scalar.activation(yo_t[:, hh, c0:c0 + cn],
                                                 yp[:, :cn], COPY)
                        else:
                            nc.vector.tensor_copy(yo_t[:, hh, c0:c0 + cn],
                                                  yp[:, :cn])
                nc.gpsimd.dma_start(out=ygf[:, :, off:off + C],
                                    in_=yo_t[:, :, :C])
    nc.finalize()
    return nc


def _get_router():
    if "router" not in _program_cache:
        _program_cache["router"] = build_router()
    return _program_cache["router"]


def _get_experts(CS):
    key = ("experts", tuple(CS))
    if key not in _program_cache:
        _program_cache[key] = build_experts(CS)
    return _program_cache[key]


def prep_router_inputs(x):
    xT = np.ascontiguousarray(x.T)
    return xT


def route_on_host(combine):
    """Per-expert token lists + load-sorted (core, slot) assignment.
    Returns (idx, assign, CS) where assign[c][s] = expert id and CS[s] is
    slot s's capacity (max load over its 8 experts, rounded up to 32)."""
    idx = [np.nonzero(combine[:, e])[0] for e in range(E)]
    loads = np.array([len(ii) for ii in idx])
    order = np.argsort(-loads, kind="stable")
    assign = [[-1] * SLOTS for _ in range(NCORES)]
    CS = []
    for s in range(SLOTS):
        grp = order[s * NCORES:(s + 1) * NCORES]
        for c in range(NCORES):
            assign[c][s] = int(grp[c])
        CS.append(max(16, int(math.ceil(loads[grp].max() / 16.0)) * 16))
    return idx, assign, tuple(CS)


def prep_expert_inputs(x, combine, idx, assign, CS, w_gate, w_up, w_down):
    """Build per-core in_maps with tile-exact bf16 layouts."""
    CT = sum(CS)
    offs = [0]
    for c in CS[:-1]:
        offs.append(offs[-1] + c)
    in_maps = []
    for c in range(NCORES):
        xg = np.zeros((128, KH, CT), NP_BF16)
        cwm = np.zeros((1, CT), NP_BF16)
        wgu = np.empty((SLOTS, 128, KI, 2, KH, 128), NP_BF16)
        wdh = np.empty((SLOTS, 128, KH, KI, 128), NP_BF16)
        for s in range(SLOTS):
            e = assign[c][s]
            off = offs[s]
            ii = idx[e]
            n = len(ii)
            if n:
                # [n, H] -> [n, KH, 128] -> [128, KH, n]
                xe = x[ii].reshape(n, KH, 128).transpose(2, 1, 0)
                xg[:, :, off:off + n] = xe.astype(NP_BF16)
                cwm[0, off:off + n] = combine[ii, e].astype(NP_BF16)
            g = w_gate[e].reshape(KI, 128, KH, 128)   # (m, i, k, p)
            u = w_up[e].reshape(KI, 128, KH, 128)
            wgu[s, :, :, 0] = g.transpose(3, 0, 2, 1).astype(NP_BF16)
            wgu[s, :, :, 1] = u.transpose(3, 0, 2, 1).astype(NP_BF16)
            d = w_down[e].reshape(KH, 128, KI, 128)   # (h, o, k, p)
            wdh[s] = d.transpose(3, 0, 2, 1).astype(NP_BF16)  # (p, h, k, o)
        in_maps.append({"xg": xg, "wgu": wgu, "wd": wdh, "cw": cwm})
    return in_maps


def unpack_expert_outputs(results, idx, assign, CS):
    """Scatter-add per-expert outputs into y [T, H] (unique indices per
    expert)."""
    offs = [0]
    for c in CS[:-1]:
        offs.append(offs[-1] + c)
    y = np.zeros((T, H), np.float32)
    for c in range(NCORES):
        ygc = results[c]["yg"]             # [128, KH, CT] bf16
        for s in range(SLOTS):
            e = assign[c][s]
            ii = idx[e]
            n = len(ii)
            if n:
                yt = ygc[:, :, offs[s]:offs[s] + n].astype(np.float32)
                # [128(p), KH(h), n] -> [H, n]: H index = h*128 + p
                y[ii] += yt.transpose(1, 0, 2).reshape(H, n).T
    return y


def kernel(hidden_states, w_router, w_gate, w_up, w_down):
    x = np.ascontiguousarray(np.asarray(hidden_states, np.float32)).reshape(T, H)
    w_gate = np.asarray(w_gate, np.float32)
    w_up = np.asarray(w_up, np.float32)
    w_down = np.asarray(w_down, np.float32)
    xT = prep_router_inputs(x)
    wrT = np.ascontiguousarray(np.asarray(w_router, np.float32).T)   # [H, E]

    # ---- Phase 1: router on device ----
    nc1 = _get_router()
    in_maps1 = [
        {"xTc": np.ascontiguousarray(xT[:, c * TPC:(c + 1) * TPC]), "wrT": wrT}
        for c in range(NCORES)
    ]
    r1 = run_bass_kernel_spmd(nc1, in_maps1, list(range(NCORES)))
    # comb layout [128, NT, E], token index = t*128 + p
    combine = np.concatenate(
        [r1.results[c]["comb"].transpose(1, 0, 2).reshape(TPC, E)
         for c in range(NCORES)], axis=0)

    # ---- Host: compaction (data movement only) ----
    idx, assign, CS = route_on_host(combine)
    in_maps2 = prep_expert_inputs(x, combine, idx, assign, CS,
                                  w_gate, w_up, w_down)

    # ---- Phase 2: expert MLPs on device ----
    nc2 = _get_experts(CS)
    r2 = run_bass_kernel_spmd(nc2, in_maps2, list(range(NCORES)))

    # ---- Host: scatter-add ----
    y = unpack_expert_outputs([r2.results[c] for c in range(NCORES)],
                              idx, assign, CS)
    return y.reshape(B, S, H)


# revision 13
# speedup vs baseline: 1.0428x; 1.0428x over previous
"""MoE (MiMoV2 FlashMoE) Trainium2 kernel: expert-parallel over 8 NeuronCores.

Strategy:
  Phase 1 (device): router — logits = x @ w_router.T computed in fp32,
    top-4 selection via exact max/mask iterations on logits, combine
    weights = sigmoid(logit) normalized over the selected 4. Each core
    handles T/8 = 512 tokens. Output: dense combine matrix [T, E]
    (4 nonzeros per row).
  Host: compaction — per-expert token index lists from combine > 0
    (pure data movement). Experts are assigned to (core, slot) by
    descending load so that slot s has capacity C_s = max load of its
    8 experts; per-core padded columns drop from 4*Cmax to sum(C_s).
  Phase 2 (device): experts — 4 expert slots per core, all matmuls in
    bf16 (1 cycle/row on PE, half the DMA traffic of f32).
    G = Wg @ X, U = Wu @ X, h = silu(G)*U*combine (Act+DVE, bf16),
    Y^T = Wd @ h, PSUM f32 -> bf16 staging -> one output DMA per slot.
  Host: scatter-add per-expert outputs into y [T, H] (unique indices
    per expert).
"""
import math
import numpy as np
from contextlib import ExitStack

import ml_dtypes

import concourse.bass as bass
import concourse.mybir as mybir
import concourse.tile as tile
from concourse import bacc
from concourse.bass_utils import run_bass_kernel_spmd

F32 = mybir.dt.float32
F32R = mybir.dt.float32r
BF16 = mybir.dt.bfloat16
NP_BF16 = ml_dtypes.bfloat16

# Problem shapes (hardcoded per contract)
E = 32          # experts
TOPK = 4
H = 1024        # hidden
I = 768         # intermediate
B, S = 2, 2048
T = B * S       # 4096 tokens
NCORES = 8
SLOTS = E // NCORES  # expert slots per core = 4
TPC = T // NCORES    # router tokens per core = 512
KH = H // 128        # 8 contraction chunks over H
KI = I // 128        # 6 contraction chunks over I

_program_cache = {}


def _ctiles(C):
    """Split C into near-equal column tiles, each <= 512 (PSUM bank).
    Measured on HW: near-equal halves beat [512, remainder] tiling."""
    n = max(1, math.ceil(C / 512))
    base = C // n
    rem = C - base * n
    sizes = [base + (1 if i < rem else 0) for i in range(n)]
    out, off = [], 0
    for s in sizes:
        out.append((off, s))
        off += s
    return out


def build_router(reps=1):
    """Per-core: logits^T = w_router @ x^T via PE (weights stationary, 512
    tokens moving, exact fp32), Act copy PSUM->SBUF, 4 batched DVE 32x32
    block-transpose instructions to [tokens, NT, E], then a batched top-4 +
    combine-weight computation on a single [128, 4, E] tile."""
    nc = bacc.Bacc()
    NT = TPC // 128  # 4 token tiles
    xTc = nc.dram_tensor("xTc", [H, TPC], F32, kind="ExternalInput")
    wrT = nc.dram_tensor("wrT", [H, E], F32, kind="ExternalInput")
    comb_out = nc.dram_tensor("comb", [128, NT, E], F32, kind="ExternalOutput")
    with ExitStack() as ctx:
        tc = ctx.enter_context(tile.TileContext(nc))
        sb = ctx.enter_context(tc.tile_pool(name="sb", bufs=1))
        work = ctx.enter_context(tc.tile_pool(name="work", bufs=2))
        ps = ctx.enter_context(tc.tile_pool(name="ps", bufs=2, space="PSUM"))

        xr = sb.tile([128, KH, TPC], F32)
        wr = sb.tile([128, KH, E], F32)
        for k in range(KH):
            nc.sync.dma_start(out=xr[:, k, :], in_=xTc[k * 128:(k + 1) * 128, :])
            nc.scalar.dma_start(out=wr[:, k, :], in_=wrT[k * 128:(k + 1) * 128, :])

        for _ in range(reps):
            lgT_p = ps.tile([E, NT, 128], F32, tag="lg", name="lgT_p")
            for k in range(KH):
                nc.tensor.matmul(lgT_p, wr[:, k, :], xr[:, k, :],
                                 start=(k == 0), stop=(k == KH - 1))
            lgT = work.tile([E, NT, 128], F32)
            nc.scalar.activation(lgT, lgT_p, mybir.ActivationFunctionType.Copy)
            # transpose to [128, NT, E]: 4 multi-block DVE transposes, one
            # per 32-partition destination group (NT 32x32 blocks each)
            lt = work.tile([128, NT, E], F32)
            for jb in range(128 // 32):
                nc.vector.transpose(
                    lt[jb * 32:(jb + 1) * 32, :, :],
                    lgT[:, :, jb * 32:(jb + 1) * 32])
            # batched top-4: find 4th max per token via iterative masking
            cur = work.tile([128, NT, E], F32)
            nc.vector.tensor_copy(cur, lt)
            m = work.tile([128, NT, 1], F32)
            ge = work.tile([128, NT, E], F32)
            for _k in range(TOPK - 1):
                nc.vector.reduce_max(m, cur, axis=mybir.AxisListType.X)
                nc.vector.tensor_tensor(ge, cur, m.broadcast_to((128, NT, E)),
                                        op=mybir.AluOpType.is_ge)
                nc.vector.scalar_tensor_tensor(cur, ge, -1e30, cur,
                                               op0=mybir.AluOpType.mult,
                                               op1=mybir.AluOpType.add)
            nc.vector.reduce_max(m, cur, axis=mybir.AxisListType.X)
            # sel = (logits >= 4th max), combine = sel*sigmoid normalized
            sel = work.tile([128, NT, E], F32)
            nc.vector.tensor_tensor(sel, lt, m.broadcast_to((128, NT, E)),
                                    op=mybir.AluOpType.is_ge)
            sig = work.tile([128, NT, E], F32)
            nc.scalar.activation(sig, lt, mybir.ActivationFunctionType.Sigmoid)
            wsel = work.tile([128, NT, E], F32)
            nc.vector.tensor_mul(wsel, sel, sig)
            ssum = work.tile([128, NT, 1], F32)
            nc.vector.reduce_sum(ssum, wsel, axis=mybir.AxisListType.X)
            # no +1e-20: top-4 sigmoids are >= sigmoid(-|logit|max) >> 0
            rsum = work.tile([128, NT, 1], F32)
            nc.vector.reciprocal(rsum, ssum)
            ct = work.tile([128, NT, E], F32)
            nc.vector.tensor_tensor(ct, wsel, rsum.broadcast_to((128, NT, E)),
                                    op=mybir.AluOpType.mult)
            nc.sync.dma_start(out=comb_out[:], in_=ct)
    nc.finalize()
    return nc


def build_experts(CS, reps=1):
    """Expert MLP kernel, bf16. CS = per-slot capacities (build constants).
    Per-core inputs (tile-exact layouts, all contiguous DMA):
      xg  [128, KH, CT]             bf16  xg[p,k,off_s+c] = x[tok_{s,c}, k*128+p]
      wgu [SLOTS, 128, KI, 2, KH, 128] bf16
          wgu[s,p,m,0,k,i] = w_gate[e_s, m*128+i, k*128+p]; [..,1,..] = w_up
      wd  [SLOTS, 128, KH, KI, 128] bf16  wd[s,p,h,k,o] = w_down[e_s, h*128+o, k*128+p]
      cw  [1, CT]                   bf16  combine weights (0 on padding)
    Output: yg [128, KH, CT] bf16, yg[p,h,off_s+c] = y^T[h*128+p, c]
    (combine-weighted, transposed)."""
    CS = tuple(CS)
    CT = sum(CS)
    CMAX = max(CS)
    offs = [0]
    for c in CS[:-1]:
        offs.append(offs[-1] + c)

    nc = bacc.Bacc()
    xgf = nc.dram_tensor("xg", [128, KH, CT], BF16, kind="ExternalInput")
    wgu = nc.dram_tensor("wgu", [SLOTS, 128, KI, 2, KH, 128], BF16,
                         kind="ExternalInput")
    wdd = nc.dram_tensor("wd", [SLOTS, 128, KH, KI, 128], BF16,
                         kind="ExternalInput")
    cwf = nc.dram_tensor("cw", [1, CT], BF16, kind="ExternalInput")
    ygf = nc.dram_tensor("yg", [128, KH, CT], BF16, kind="ExternalOutput")
    warm_out = nc.dram_tensor("warm", [128, 1], F32, kind="ExternalOutput")

    with ExitStack() as ctx:
        tc = ctx.enter_context(tile.TileContext(nc))
        const = ctx.enter_context(tc.tile_pool(name="const", bufs=1))
        xgp = ctx.enter_context(tc.tile_pool(name="xgp", bufs=2))
        wgp = ctx.enter_context(tc.tile_pool(name="wgp", bufs=2))
        wdp = ctx.enter_context(tc.tile_pool(name="wdp", bufs=2))
        hp = ctx.enter_context(tc.tile_pool(name="hp", bufs=2))
        msc = ctx.enter_context(tc.tile_pool(name="msc", bufs=3))
        outp = ctx.enter_context(tc.tile_pool(name="outp", bufs=2))
        ps_gu = ctx.enter_context(tc.tile_pool(name="ps_gu", bufs=2, space="PSUM"))
        ps_d = ctx.enter_context(tc.tile_pool(name="ps_d", bufs=2, space="PSUM"))

        cwb = const.tile([128, CT], BF16, tag="cw")
        nc.scalar.dma_start(out=cwb, in_=cwf[0:1, :].partition_broadcast(128))

        # PE warm-up: keep TensorE busy while the first weight/activation
        # DMAs land, so the HAM clock-gate releases (1.2 -> 2.4 GHz) before
        # real matmuls start. Results are dumped to a debug output.
        wtile = const.tile([128, 512], F32R, tag="warm")
        nc.vector.memset(wtile.bitcast(F32), 0.0)
        wps = ps_d.tile([128, 512], F32, tag="warmp")
        for wi in range(6):
            nc.tensor.matmul(wps, wtile[:, :128], wtile,
                             start=(wi == 0), stop=(wi == 5))
        wres = const.tile([128, 1], F32, tag="warmres")
        nc.vector.tensor_copy(wres, wps[:, 0:1])
        nc.gpsimd.dma_start(out=warm_out[:], in_=wres)

        SILU = mybir.ActivationFunctionType.Silu
        COPY = mybir.ActivationFunctionType.Copy
        for _ in range(reps):
            for s in range(SLOTS):
                C, off = CS[s], offs[s]
                cts = _ctiles(C)
                xg_t = xgp.tile([128, KH, CMAX], BF16, tag="xg")
                wgu_t = wgp.tile([128, KI, 2, KH, 128], BF16, tag="wgu")
                wd_t = wdp.tile([128, KH, KI, 128], BF16, tag="wd")
                nc.sync.dma_start(out=wgu_t[:, :, 0], in_=wgu[s, :, :, 0])
                nc.scalar.dma_start(out=wgu_t[:, :, 1], in_=wgu[s, :, :, 1])
                nc.sync.dma_start(out=xg_t[:, :, :C], in_=xgf[:, :, off:off + C])
                nc.scalar.dma_start(out=wd_t, in_=wdd[s])

                h_t = hp.tile([128, KI, CMAX], BF16, tag="h")
                for m in range(KI):
                    for ci, (c0, cn) in enumerate(cts):
                        gp = ps_gu.tile([128, 512], F32, tag="gp")
                        for k in range(KH):
                            nc.tensor.matmul(gp[:, :cn], wgu_t[:, m, 0, k, :],
                                             xg_t[:, k, c0:c0 + cn],
                                             start=(k == 0), stop=(k == KH - 1))
                        up = ps_gu.tile([128, 512], F32, tag="up")
                        for k in range(KH):
                            nc.tensor.matmul(up[:, :cn], wgu_t[:, m, 1, k, :],
                                             xg_t[:, k, c0:c0 + cn],
                                             start=(k == 0), stop=(k == KH - 1))
                        sg = msc.tile([128, 512], BF16, tag="sg")
                        nc.scalar.activation(sg[:, :cn], gp[:, :cn], SILU)
                        t1 = msc.tile([128, 512], BF16, tag="t1")
                        nc.vector.tensor_mul(t1[:, :cn], sg[:, :cn], up[:, :cn])
                        nc.vector.tensor_mul(h_t[:, m, c0:c0 + cn], t1[:, :cn],
                                             cwb[:, off + c0:off + c0 + cn])
                yo_t = outp.tile([128, KH, CMAX], BF16, tag="yo")
                for hh in range(KH):
                    for ci, (c0, cn) in enumerate(cts):
                        yp = ps_d.tile([128, 512], F32, tag="yp")
                        for k in range(KI):
                            nc.tensor.matmul(yp[:, :cn], wd_t[:, hh, k, :],
                                             h_t[:, k, c0:c0 + cn],
                                             start=(k == 0), stop=(k == KI - 1))
                        # alternate the PSUM->SBUF bf16 copy between Act/DVE
                        if (hh * len(cts) + ci) % 2 == 0:
                            nc.scalar.activation(yo_t[:, hh, c0:c0 + cn],
                                                 yp[:, :cn], COPY)
                        else:
                            nc.vector.tensor_copy(yo_t[:, hh, c0:c0 + cn],
                                                  yp[:, :cn])
                nc.gpsimd.dma_start(out=ygf[:, :, off:off + C],
                                    in_=yo_t[:, :, :C])
    nc.finalize()
    return nc


def _get_router():
    if "router" not in _program_cache:
        _program_cache["router"] = build_router()
    return _program_cache["router"]


def _get_experts(CS):
    key = ("experts", tuple(CS))
    if key not in _program_cache:
        _program_cache[key] = build_experts(CS)
    return _program_cache[key]


def prep_router_inputs(x):
    xT = np.ascontiguousarray(x.T)
    return xT


def route_on_host(combine):
    """Per-expert token lists + load-sorted (core, slot) assignment.
    Returns (idx, assign, CS) where assign[c][s] = expert id and CS[s] is
    slot s's capacity (max load over its 8 experts, rounded up to 32)."""
    idx = [np.nonzero(combine[:, e])[0] for e in range(E)]
    loads = np.array([len(ii) for ii in idx])
    order = np.argsort(-loads, kind="stable")
    assign = [[-1] * SLOTS for _ in range(NCORES)]
    CS = []
    for s in range(SLOTS):
        grp = order[s * NCORES:(s + 1) * NCORES]
        for c in range(NCORES):
            assign[c][s] = int(grp[c])
        CS.append(max(16, int(math.ceil(loads[grp].max() / 16.0)) * 16))
    return idx, assign, tuple(CS)


def prep_expert_inputs(x, combine, idx, assign, CS, w_gate, w_up, w_down):
    """Build per-core in_maps with tile-exact bf16 layouts."""
    CT = sum(CS)
    offs = [0]
    for c in CS[:-1]:
        offs.append(offs[-1] + c)
    in_maps = []
    for c in range(NCORES):
        xg = np.zeros((128, KH, CT), NP_BF16)
        cwm = np.zeros((1, CT), NP_BF16)
        wgu = np.empty((SLOTS, 128, KI, 2, KH, 128), NP_BF16)
        wdh = np.empty((SLOTS, 128, KH, KI, 128), NP_BF16)
        for s in range(SLOTS):
            e = assign[c][s]
            off = offs[s]
            ii = idx[e]
            n = len(ii)
            if n:
                # [n, H] -> [n, KH, 128] -> [128, KH, n]
                xe = x[ii].reshape(n, KH, 128).transpose(2, 1, 0)
                xg[:, :, off:off + n] = xe.astype(NP_BF16)
                cwm[0, off:off + n] = combine[ii, e].astype(NP_BF16)
            g = w_gate[e].reshape(KI, 128, KH, 128)   # (m, i, k, p)
            u = w_up[e].reshape(KI, 128, KH, 128)
            wgu[s, :, :, 0] = g.transpose(3, 0, 2, 1).astype(NP_BF16)
            wgu[s, :, :, 1] = u.transpose(3, 0, 2, 1).astype(NP_BF16)
            d = w_down[e].reshape(KH, 128, KI, 128)   # (h, o, k, p)
            wdh[s] = d.transpose(3, 0, 2, 1).astype(NP_BF16)  # (p, h, k, o)
        in_maps.append({"xg": xg, "wgu": wgu, "wd": wdh, "cw": cwm})
    return in_maps


def unpack_expert_outputs(results, idx, assign, CS):
    """Scatter-add per-expert outputs into y [T, H] (unique indices per
    expert)."""
    offs = [0]
    for c in CS[:-1]:
        offs.append(offs[-1] + c)
    y = np.zeros((T, H), np.float32)
    for c in range(NCORES):
        ygc = results[c]["yg"]             # [128, KH, CT] bf16
        for s in range(SLOTS):
            e = assign[c][s]
            ii = idx[e]
            n = len(ii)
            if n:
                yt = ygc[:, :, offs[s]:offs[s] + n].astype(np.float32)
                # [128(p), KH(h), n] -> [H, n]: H index = h*128 + p
                y[ii] += yt.transpose(1, 0, 2).reshape(H, n).T
    return y


def kernel(hidden_states, w_router, w_gate, w_up, w_down):
    x = np.ascontiguousarray(np.asarray(hidden_states, np.float32)).reshape(T, H)
    w_gate = np.asarray(w_gate, np.float32)
    w_up = np.asarray(w_up, np.float32)
    w_down = np.asarray(w_down, np.float32)
    xT = prep_router_inputs(x)
    wrT = np.ascontiguousarray(np.asarray(w_router, np.float32).T)   # [H, E]

    # ---- Phase 1: router on device ----
    nc1 = _get_router()
    in_maps1 = [
        {"xTc": np.ascontiguousarray(xT[:, c * TPC:(c + 1) * TPC]), "wrT": wrT}
        for c in range(NCORES)
    ]
    r1 = run_bass_kernel_spmd(nc1, in_maps1, list(range(NCORES)))
    # comb layout [128, NT, E], token index = t*128 + p
    combine = np.concatenate(
        [r1.results[c]["comb"].transpose(1, 0, 2).reshape(TPC, E)
         for c in range(NCORES)], axis=0)

    # ---- Host: compaction (data movement only) ----
    idx, assign, CS = route_on_host(combine)
    in_maps2 = prep_expert_inputs(x, combine, idx, assign, CS,
                                  w_gate, w_up, w_down)

    # ---- Phase 2: expert MLPs on device ----
    nc2 = _get_experts(CS)
    r2 = run_bass_kernel_spmd(nc2, in_maps2, list(range(NCORES)))

    # ---- Host: scatter-add ----
    y = unpack_expert_outputs([r2.results[c] for c in range(NCORES)],
                              idx, assign, CS)
    return y.reshape(B, S, H)
